# revision 5
# baseline (speedup 1.0000x reference)
"""Sparse expert-parallel MoE kernel for Trainium2 (8 NeuronCores).

Strategy (hardcoded for the nn_MoE problem: H=1024, E=8, top-k=2, I=1408,
shared-I=2816, T=2*2048=4096 tokens, f32 inputs):

- The gate (softmax top-2) is tiny (0.03% of FLOPs) and is evaluated on the
  host in float64; routing decisions match the f32 reference (min rank-2/3
  score gap for this problem's data is ~4e-5, far above f32 noise).
- Routed experts are EXPERT-PARALLEL with true top-2 sparsity: core r owns
  expert r and computes it only over the tokens routed to it (host-side
  gather -> padded capacity C, a multiple of 64).  This is ~4x fewer FLOPs
  than dense all-expert compute.
- The shared expert is sharded 4x2: cores are split into 4 token-groups of
  2; within a group each core owns a 1408-wide half of the 2816 shared
  intermediate dim (11 full 128-tiles -> no partial-tile waste).  Partials
  are summed on the host.
- Combine: host scatter-adds  w_e(t) * expert_e(x_t)  (f32) plus the shared
  partial sums.  No on-device collectives.
- All matmuls run in bf16 with f32 PSUM accumulation (host pre-casts).
- The shared phase runs FIRST (its up-weights are DMA'd in interleaved
  128x128 block order so the first matmul group only waits for ~0.5 MB);
  the routed phase ends with a tiny 64-token chunk so the drain tail is
  short.

Layouts put features on the partition axis and tokens on the free axis:
    up:   hg[i, t] = sum_h wg[h, i] * xT[h, t]   (lhsT=wg nat., rhs=xT)
    down: eo[h, t] = sum_i wd[i, h] * act[i, t]  (lhsT=wd nat., rhs=act)
"""

import os
import sys

for _p in ("/opt/trn_rl_repo", "/root/.axon_site/_ro/trn_rl_repo"):
    if os.path.isdir(_p) and _p not in sys.path:
        sys.path.insert(0, _p)

import numpy as np

import concourse.bass as bass
import concourse.mybir as mybir
import concourse.tile as tile
from concourse import bacc
from concourse.bass_utils import run_bass_kernel_spmd

F32 = mybir.dt.float32
BF16 = mybir.dt.bfloat16
BF16_NP = mybir.dt.np(mybir.dt.bfloat16)
AX = mybir.AxisListType
ALU = mybir.AluOpType
ACTF = mybir.ActivationFunctionType

H = 1024           # hidden
E = 8              # experts = cores
TOP_K = 2
I_R = 1408         # routed intermediate = shared intermediate half (2816/2)
TS = 1024          # shared-expert tokens per core (4096 / 4 groups)
N_CORES = 8
KC = H // 128      # 8 contraction chunks over hidden
IT_R = I_R // 128  # 11 intermediate tiles (routed and shared-half alike)
TC = 512           # token tile (PSUM bank = 512 f32)

LAST_RESULT = None  # BassKernelResults of the most recent run (for profiling)


def _chunks_of(n):
    out = [TC] * (n // TC)
    if n % TC:
        out.append(n % TC)
    return out


def build_nc(C, trace_sim=False, silu_via_sigmoid=False):
    """Build the SPMD Bass program (identical on all 8 cores).

    C: routed-token capacity per core (multiple of 64).
    silu_via_sigmoid: CoreSim has no Silu LUT; emulate as x*sigmoid(x).
    """
    nc = bacc.Bacc("TRN2", target_bir_lowering=False, debug=False,
                   num_devices=N_CORES)

    xr = nc.dram_tensor("xr", [H, C], BF16, kind="ExternalInput")
    xs = nc.dram_tensor("xs", [H, TS], BF16, kind="ExternalInput")
    wg = nc.dram_tensor("wg", [H, I_R], BF16, kind="ExternalInput")
    wu = nc.dram_tensor("wu", [H, I_R], BF16, kind="ExternalInput")
    wd = nc.dram_tensor("wd", [I_R, H], BF16, kind="ExternalInput")
    # shared up-weights in interleaved [it, k, 128, 128] block layout
    sg = nc.dram_tensor("sg", [IT_R * KC * 128, 128], BF16,
                        kind="ExternalInput")
    su = nc.dram_tensor("su", [IT_R * KC * 128, 128], BF16,
                        kind="ExternalInput")
    sd = nc.dram_tensor("sd", [I_R, H], BF16, kind="ExternalInput")
    yr = nc.dram_tensor("yr", [H, C], F32, kind="ExternalOutput")
    ys = nc.dram_tensor("ys", [H, TS], F32, kind="ExternalOutput")

    with tile.TileContext(nc, trace_sim=trace_sim) as tc:
        with (
            tc.tile_pool(name="const", bufs=1) as cpool,
            tc.tile_pool(name="xin", bufs=2) as xpool,
            tc.tile_pool(name="act", bufs=2) as actpool,
            tc.tile_pool(name="tmp", bufs=3) as tpool,
            tc.tile_pool(name="eo", bufs=3) as eopool,
            tc.tile_pool(name="ps_up", bufs=4, space="PSUM") as ps_up,
            tc.tile_pool(name="ps_o", bufs=2, space="PSUM") as ps_o,
        ):
            # ---- shared-phase inputs first, in consumption order, so the
            # PE can start after ~0.5 MB of DMA ----
            xs0 = xpool.tile([128, KC, TC], BF16, tag="x")
            for k in range(KC):
                nc.sync.dma_start(xs0[:, k, :], xs[k * 128:(k + 1) * 128, 0:TC])
            sg_ts, su_ts = [], []
            for it in range(IT_R):
                sgt = cpool.tile([128, KC, 128], BF16, tag=f"sg{it}")
                sut = cpool.tile([128, KC, 128], BF16, tag=f"su{it}")
                for k in range(KC):
                    r0 = (it * KC + k) * 128
                    nc.sync.dma_start(sgt[:, k, :], sg[r0:r0 + 128, :])
                for k in range(KC):
                    r0 = (it * KC + k) * 128
                    nc.sync.dma_start(sut[:, k, :], su[r0:r0 + 128, :])
                sg_ts.append(sgt)
                su_ts.append(sut)
            sd_ts = []
            for it in range(IT_R):
                sdt = cpool.tile([128, H], BF16, tag=f"sd{it}")
                nc.sync.dma_start(sdt[:, :], sd[it * 128:(it + 1) * 128, :])
                sd_ts.append(sdt)
            # routed weights (needed ~100us in; stream behind shared ones)
            wg_ks, wu_ks = [], []
            for k in range(KC):
                wgk = cpool.tile([128, I_R], BF16, tag=f"wg{k}")
                nc.sync.dma_start(wgk[:, :], wg[k * 128:(k + 1) * 128, :])
                wuk = cpool.tile([128, I_R], BF16, tag=f"wu{k}")
                nc.sync.dma_start(wuk[:, :], wu[k * 128:(k + 1) * 128, :])
                wg_ks.append(wgk)
                wu_ks.append(wuk)
            wd_ts = []
            for it in range(IT_R):
                wdt = cpool.tile([128, H], BF16, tag=f"wd{it}")
                nc.sync.dma_start(wdt[:, :], wd[it * 128:(it + 1) * 128, :])
                wd_ts.append(wdt)

            def swiglu_chunk(x_t, n, gate_f, up_f, act_t):
                """act[i, :n] = silu(gate) * up over this token chunk.

                gate_f/up_f: (it, k) -> lhsT [128, 128] weight block.
                """
                for it in range(IT_R):
                    pg = ps_up.tile([128, TC], F32, tag="up")
                    for k in range(KC):
                        nc.tensor.matmul(pg[:, :n], gate_f(it, k),
                                         x_t[:, k, :n],
                                         start=(k == 0), stop=(k == KC - 1))
                    pu = ps_up.tile([128, TC], F32, tag="up")
                    for k in range(KC):
                        nc.tensor.matmul(pu[:, :n], up_f(it, k),
                                         x_t[:, k, :n],
                                         start=(k == 0), stop=(k == KC - 1))
                    sa = tpool.tile([128, TC], F32, tag="sa")
                    if silu_via_sigmoid:
                        nc.scalar.activation(sa[:, :n], pg[:, :n],
                                             ACTF.Sigmoid)
                        nc.vector.tensor_mul(sa[:, :n], sa[:, :n], pg[:, :n])
                    else:
                        nc.scalar.activation(sa[:, :n], pg[:, :n], ACTF.Silu)
                    nc.vector.tensor_mul(act_t[:, it, :n], sa[:, :n],
                                         pu[:, :n])

            def down_chunk(act_t, n, down_ts, out_d, t0):
                for hc in range(KC):
                    h0 = hc * 128
                    po = ps_o.tile([128, TC], F32, tag="o")
                    for it in range(IT_R):
                        nc.tensor.matmul(
                            po[:, :n], down_ts[it][:, h0:h0 + 128],
                            act_t[:, it, :n], start=(it == 0),
                            stop=(it == IT_R - 1))
                    eo = eopool.tile([128, TC], F32)
                    nc.vector.tensor_copy(eo[:, :n], po[:, :n])
                    nc.sync.dma_start(out_d[h0:h0 + 128, t0:t0 + n], eo[:, :n])

            # ---- shared expert half over this core's token group ----
            t0 = 0
            for ci, n in enumerate(_chunks_of(TS)):
                if ci == 0:
                    x_t = xs0
                else:
                    x_t = xpool.tile([128, KC, TC], BF16, tag="x")
                    for k in range(KC):
                        nc.sync.dma_start(
                            x_t[:, k, :n], xs[k * 128:(k + 1) * 128, t0:t0 + n])
                act_t = actpool.tile([128, IT_R, TC], BF16, tag="act")
                swiglu_chunk(x_t, n, lambda it, k: sg_ts[it][:, k, :],
                             lambda it, k: su_ts[it][:, k, :], act_t)
                down_chunk(act_t, n, sd_ts, ys, t0)
                t0 += n

            # ---- routed expert over gathered tokens (64-col tail last) ----
            t0 = 0
            for n in _chunks_of(C):
                x_t = xpool.tile([128, KC, TC], BF16, tag="x")
                for k in range(KC):
                    nc.sync.dma_start(
                        x_t[:, k, :n], xr[k * 128:(k + 1) * 128, t0:t0 + n])
                act_t = actpool.tile([128, IT_R, TC], BF16, tag="act")
                swiglu_chunk(
                    x_t, n,
                    lambda it, k: wg_ks[k][:, it * 128:(it + 1) * 128],
                    lambda it, k: wu_ks[k][:, it * 128:(it + 1) * 128], act_t)
                down_chunk(act_t, n, wd_ts, yr, t0)
                t0 += n

    nc.compile()
    return nc


def _route_host(xf, gate_w):
    """Replicate the reference MoEGate exactly (float64 for determinism)."""
    logits = xf.astype(np.float64) @ gate_w.astype(np.float64).T
    m = logits.max(axis=-1, keepdims=True)
    ex = np.exp(logits - m)
    sc = ex / ex.sum(axis=-1, keepdims=True)
    topi = np.argsort(-sc, axis=-1, kind="stable")[:, :TOP_K]   # ties: low idx
    topw = np.take_along_axis(sc, topi, axis=-1)
    topw = topw / (topw.sum(axis=-1, keepdims=True) + 1e-20)    # SCALE = 1.0
    return topi, topw


def _pack_blocks(w):
    """[H, I_R] -> interleaved [it, k, 128, 128] block layout, flattened."""
    a = np.ascontiguousarray(w).astype(BF16_NP)
    a = a.reshape(KC, 128, IT_R, 128).transpose(2, 0, 1, 3)
    return np.ascontiguousarray(a.reshape(IT_R * KC * 128, 128))


_NC_CACHE = {}


def kernel(x, gate_w, wg, wu, wd, swg, swu, swd):
    global LAST_RESULT
    x = np.asarray(x, np.float32)
    B, S, _ = x.shape
    T = B * S
    xf = x.reshape(T, H)

    # ---- host gate + dispatch ----
    topi, topw = _route_host(xf, np.asarray(gate_w, np.float32))
    e_ids = topi.ravel()
    t_ids = np.repeat(np.arange(T), TOP_K)
    w_all = topw.ravel()
    order = np.argsort(e_ids, kind="stable")
    e_sorted = e_ids[order]
    t_sorted = t_ids[order]
    w_sorted = w_all[order]
    counts = np.bincount(e_sorted, minlength=E)
    starts = np.concatenate([[0], np.cumsum(counts)])
    C = max(64, int(-(-counts.max() // 64)) * 64)

    if C not in _NC_CACHE:
        _NC_CACHE[C] = build_nc(C)
    nc = _NC_CACHE[C]

    xfT_bf = np.ascontiguousarray(xf.T).astype(BF16_NP)   # [H, T]
    wg = np.asarray(wg, np.float32)
    wu = np.asarray(wu, np.float32)
    wd = np.asarray(wd, np.float32)
    swg = np.asarray(swg, np.float32)
    swu = np.asarray(swu, np.float32)
    swd = np.asarray(swd, np.float32)

    in_maps = []
    idx_r = []
    w_r = []
    for r in range(N_CORES):
        lo, hi = starts[r], starts[r + 1]
        idx = t_sorted[lo:hi]
        idx_r.append(idx)
        w_r.append(w_sorted[lo:hi])
        xr = np.zeros((H, C), dtype=BF16_NP)
        xr[:, :len(idx)] = xfT_bf[:, idx]
        g, q = divmod(r, 2)
        in_maps.append({
            "xr": xr,
            "xs": np.ascontiguousarray(xfT_bf[:, g * TS:(g + 1) * TS]),
            "wg": np.ascontiguousarray(wg[r]).astype(BF16_NP),
            "wu": np.ascontiguousarray(wu[r]).astype(BF16_NP),
            "wd": np.ascontiguousarray(wd[r]).astype(BF16_NP),
            "sg": _pack_blocks(swg[:, q * I_R:(q + 1) * I_R]),
            "su": _pack_blocks(swu[:, q * I_R:(q + 1) * I_R]),
            "sd": np.ascontiguousarray(
                swd[q * I_R:(q + 1) * I_R, :]).astype(BF16_NP),
        })

    res = run_bass_kernel_spmd(nc, in_maps, core_ids=list(range(N_CORES)))
    LAST_RESULT = res

    # ---- host combine: shared partial sums + weighted routed scatter ----
    yT = np.zeros((H, T), np.float32)
    for g in range(4):
        np.add(res.results[2 * g]["ys"], res.results[2 * g + 1]["ys"],
               out=yT[:, g * TS:(g + 1) * TS])
    for r in range(N_CORES):
        n = len(idx_r[r])
        if n:
            yT[:, idx_r[r]] += (res.results[r]["yr"][:, :n]
                                * w_r[r][None, :].astype(np.float32))
    return np.ascontiguousarray(yT.T).reshape(B, S, H).astype(np.float32)


# revision 6
# speedup vs baseline: 1.3613x; 1.3613x over previous
"""Sparse expert-parallel MoE kernel for Trainium2 (8 NeuronCores).

Strategy (hardcoded for the nn_MoE problem: H=1024, E=8, top-k=2, I=1408,
shared-I=2816, T=2*2048=4096 tokens, f32 inputs):

- The gate (softmax top-2) is tiny (0.03% of FLOPs) and is evaluated on the
  host in float64; routing decisions match the f32 reference (min rank-2/3
  score gap for this problem's data is ~4e-5, far above f32 noise).
- Routed experts are EXPERT-PARALLEL with true top-2 sparsity: core r owns
  expert r and computes it only over the tokens routed to it (host-side
  gather -> padded capacity C, a multiple of 64).  This is ~4x fewer FLOPs
  than dense all-expert compute.
- The shared expert is sharded 4x2: cores are split into 4 token-groups of
  2; within a group each core owns a 1408-wide half of the 2816 shared
  intermediate dim (11 full 128-tiles -> no partial-tile waste).  Partials
  are summed on the host.
- Combine: host scatter-adds  w_e(t) * expert_e(x_t)  (f32) plus the shared
  partial sums.  No on-device collectives.
- All matmuls run in bf16 with f32 PSUM accumulation (host pre-casts).
- DMA layouts are chosen so every transfer has >=2KB contiguous bytes per
  partition (the DMA pays ~110ns per line descriptor): x inputs are packed
  [128, KC, ntok] and fully SBUF-resident; shared up-weights are packed
  [128, IT, KC*128] and DMA'd per intermediate tile in consumption order
  (first matmul group only waits for ~0.5 MB).
- The shared phase runs FIRST; the routed phase starts with its small
  64-token remainder chunk (so the inefficient small-line store overlaps
  compute) and ends on a full 512 chunk (short drain tail).

Layouts put features on the partition axis and tokens on the free axis:
    up:   hg[i, t] = sum_h wg[h, i] * xT[h, t]   (lhsT=wg nat., rhs=xT)
    down: eo[h, t] = sum_i wd[i, h] * act[i, t]  (lhsT=wd nat., rhs=act)
"""

import os
import sys

for _p in ("/opt/trn_rl_repo", "/root/.axon_site/_ro/trn_rl_repo"):
    if os.path.isdir(_p) and _p not in sys.path:
        sys.path.insert(0, _p)

import numpy as np

import concourse.bass as bass
import concourse.mybir as mybir
import concourse.tile as tile
from concourse import bacc
from concourse.bass_utils import run_bass_kernel_spmd

F32 = mybir.dt.float32
BF16 = mybir.dt.bfloat16
BF16_NP = mybir.dt.np(mybir.dt.bfloat16)
AX = mybir.AxisListType
ALU = mybir.AluOpType
ACTF = mybir.ActivationFunctionType

H = 1024           # hidden
E = 8              # experts = cores
TOP_K = 2
I_R = 1408         # routed intermediate = shared intermediate half (2816/2)
TS = 1024          # shared-expert tokens per core (4096 / 4 groups)
N_CORES = 8
KC = H // 128      # 8 contraction chunks over hidden
IT_R = I_R // 128  # 11 intermediate tiles (routed and shared-half alike)
TC = 512           # token tile (PSUM bank = 512 f32)

LAST_RESULT = None  # BassKernelResults of the most recent run (for profiling)


def _chunks_of(n, small_first=False):
    out = [TC] * (n // TC)
    if n % TC:
        if small_first:
            out.insert(0, n % TC)
        else:
            out.append(n % TC)
    return out


def build_nc(C, trace_sim=False, silu_via_sigmoid=False):
    """Build the SPMD Bass program (identical on all 8 cores).

    C: routed-token capacity per core (multiple of 64).
    silu_via_sigmoid: CoreSim has no Silu LUT; emulate as x*sigmoid(x).
    """
    nc = bacc.Bacc("TRN2", target_bir_lowering=False, debug=False,
                   num_devices=N_CORES)

    # x inputs packed [128, KC, ntok]: per-k DMA has ntok*2B contiguous/line
    xr = nc.dram_tensor("xr", [128, KC, C], BF16, kind="ExternalInput")
    xs = nc.dram_tensor("xs", [128, KC, TS], BF16, kind="ExternalInput")
    wg = nc.dram_tensor("wg", [H, I_R], BF16, kind="ExternalInput")
    wu = nc.dram_tensor("wu", [H, I_R], BF16, kind="ExternalInput")
    wd = nc.dram_tensor("wd", [I_R, H], BF16, kind="ExternalInput")
    # shared up-weights packed [128, IT, KC*128]: per-it DMA, 2KB lines
    sg = nc.dram_tensor("sg", [128, IT_R, KC * 128], BF16,
                        kind="ExternalInput")
    su = nc.dram_tensor("su", [128, IT_R, KC * 128], BF16,
                        kind="ExternalInput")
    sd = nc.dram_tensor("sd", [I_R, H], BF16, kind="ExternalInput")
    yr = nc.dram_tensor("yr", [H, C], F32, kind="ExternalOutput")
    ys = nc.dram_tensor("ys", [H, TS], F32, kind="ExternalOutput")

    with tile.TileContext(nc, trace_sim=trace_sim) as tc:
        with (
            tc.tile_pool(name="const", bufs=1) as cpool,
            tc.tile_pool(name="act", bufs=2) as actpool,
            tc.tile_pool(name="tmp", bufs=3) as tpool,
            tc.tile_pool(name="eo", bufs=3) as eopool,
            tc.tile_pool(name="ps_up", bufs=4, space="PSUM") as ps_up,
            tc.tile_pool(name="ps_o", bufs=2, space="PSUM") as ps_o,
        ):
            # ---- inputs in consumption order: shared x, then shared
            # up-weights per intermediate tile, so the PE starts after
            # ~2.5 MB of efficient DMA ----
            xs_t = cpool.tile([128, KC, TS], BF16, tag="xs")
            for k in range(KC):
                nc.sync.dma_start(xs_t[:, k, :], xs[:, k, :])
            sg_ts, su_ts = [], []
            for it in range(IT_R):
                sgt = cpool.tile([128, KC, 128], BF16, tag=f"sg{it}")
                nc.sync.dma_start(sgt[:, :, :], sg[:, it, :])
                sut = cpool.tile([128, KC, 128], BF16, tag=f"su{it}")
                nc.sync.dma_start(sut[:, :, :], su[:, it, :])
                sg_ts.append(sgt)
                su_ts.append(sut)
            sd_ts = []
            for it in range(IT_R):
                sdt = cpool.tile([128, H], BF16, tag=f"sd{it}")
                nc.sync.dma_start(sdt[:, :], sd[it * 128:(it + 1) * 128, :])
                sd_ts.append(sdt)
            # routed inputs (needed ~120us in; stream behind shared ones)
            xr_t = cpool.tile([128, KC, C], BF16, tag="xr")
            for k in range(KC):
                nc.sync.dma_start(xr_t[:, k, :], xr[:, k, :])
            wg_ks, wu_ks = [], []
            for k in range(KC):
                wgk = cpool.tile([128, I_R], BF16, tag=f"wg{k}")
                nc.sync.dma_start(wgk[:, :], wg[k * 128:(k + 1) * 128, :])
                wuk = cpool.tile([128, I_R], BF16, tag=f"wu{k}")
                nc.sync.dma_start(wuk[:, :], wu[k * 128:(k + 1) * 128, :])
                wg_ks.append(wgk)
                wu_ks.append(wuk)
            wd_ts = []
            for it in range(IT_R):
                wdt = cpool.tile([128, H], BF16, tag=f"wd{it}")
                nc.sync.dma_start(wdt[:, :], wd[it * 128:(it + 1) * 128, :])
                wd_ts.append(wdt)

            def swiglu_chunk(x_t, t0, n, gate_f, up_f, act_t):
                """act[i, :n] = silu(gate) * up over tokens [t0, t0+n).

                gate_f/up_f: (it, k) -> lhsT [128, 128] weight block.
                """
                for it in range(IT_R):
                    pg = ps_up.tile([128, TC], F32, tag="up")
                    for k in range(KC):
                        nc.tensor.matmul(pg[:, :n], gate_f(it, k),
                                         x_t[:, k, t0:t0 + n],
                                         start=(k == 0), stop=(k == KC - 1))
                    pu = ps_up.tile([128, TC], F32, tag="up")
                    for k in range(KC):
                        nc.tensor.matmul(pu[:, :n], up_f(it, k),
                                         x_t[:, k, t0:t0 + n],
                                         start=(k == 0), stop=(k == KC - 1))
                    sa = tpool.tile([128, TC], F32, tag="sa")
                    if silu_via_sigmoid:
                        nc.scalar.activation(sa[:, :n], pg[:, :n],
                                             ACTF.Sigmoid)
                        nc.vector.tensor_mul(sa[:, :n], sa[:, :n], pg[:, :n])
                    else:
                        nc.scalar.activation(sa[:, :n], pg[:, :n], ACTF.Silu)
                    nc.vector.tensor_mul(act_t[:, it, :n], sa[:, :n],
                                         pu[:, :n])

            def down_chunk(act_t, n, down_ts, out_d, t0):
                for hc in range(KC):
                    h0 = hc * 128
                    po = ps_o.tile([128, TC], F32, tag="o")
                    for it in range(IT_R):
                        nc.tensor.matmul(
                            po[:, :n], down_ts[it][:, h0:h0 + 128],
                            act_t[:, it, :n], start=(it == 0),
                            stop=(it == IT_R - 1))
                    eo = eopool.tile([128, TC], F32)
                    nc.vector.tensor_copy(eo[:, :n], po[:, :n])
                    nc.sync.dma_start(out_d[h0:h0 + 128, t0:t0 + n], eo[:, :n])

            # ---- shared expert half over this core's token group ----
            t0 = 0
            for n in _chunks_of(TS):
                act_t = actpool.tile([128, IT_R, TC], BF16, tag="act")
                swiglu_chunk(xs_t, t0, n, lambda it, k: sg_ts[it][:, k, :],
                             lambda it, k: su_ts[it][:, k, :], act_t)
                down_chunk(act_t, n, sd_ts, ys, t0)
                t0 += n

            # ---- routed expert over gathered tokens (small chunk first:
            # its small-line store DMA overlaps later compute) ----
            t0 = 0
            for n in _chunks_of(C, small_first=True):
                act_t = actpool.tile([128, IT_R, TC], BF16, tag="act")
                swiglu_chunk(
                    xr_t, t0, n,
                    lambda it, k: wg_ks[k][:, it * 128:(it + 1) * 128],
                    lambda it, k: wu_ks[k][:, it * 128:(it + 1) * 128], act_t)
                down_chunk(act_t, n, wd_ts, yr, t0)
                t0 += n

    nc.compile()
    return nc


def _route_host(xf, gate_w):
    """Replicate the reference MoEGate exactly (float64 for determinism)."""
    logits = xf.astype(np.float64) @ gate_w.astype(np.float64).T
    m = logits.max(axis=-1, keepdims=True)
    ex = np.exp(logits - m)
    sc = ex / ex.sum(axis=-1, keepdims=True)
    topi = np.argsort(-sc, axis=-1, kind="stable")[:, :TOP_K]   # ties: low idx
    topw = np.take_along_axis(sc, topi, axis=-1)
    topw = topw / (topw.sum(axis=-1, keepdims=True) + 1e-20)    # SCALE = 1.0
    return topi, topw


def _pack_x(xT_bf):
    """[H, ntok] -> [128, KC, ntok] partition-major pack."""
    n = xT_bf.shape[1]
    return np.ascontiguousarray(
        xT_bf.reshape(KC, 128, n).transpose(1, 0, 2))


def _pack_up_w(w):
    """[H, I_R] -> [128, IT, KC*128] pack (2KB lines per it-tile DMA)."""
    a = np.ascontiguousarray(w).astype(BF16_NP)
    a = a.reshape(KC, 128, IT_R, 128).transpose(1, 2, 0, 3)
    return np.ascontiguousarray(a.reshape(128, IT_R, KC * 128))


_NC_CACHE = {}


def kernel(x, gate_w, wg, wu, wd, swg, swu, swd):
    global LAST_RESULT
    x = np.asarray(x, np.float32)
    B, S, _ = x.shape
    T = B * S
    xf = x.reshape(T, H)

    # ---- host gate + dispatch ----
    topi, topw = _route_host(xf, np.asarray(gate_w, np.float32))
    e_ids = topi.ravel()
    t_ids = np.repeat(np.arange(T), TOP_K)
    w_all = topw.ravel()
    order = np.argsort(e_ids, kind="stable")
    e_sorted = e_ids[order]
    t_sorted = t_ids[order]
    w_sorted = w_all[order]
    counts = np.bincount(e_sorted, minlength=E)
    starts = np.concatenate([[0], np.cumsum(counts)])
    C = max(64, int(-(-counts.max() // 64)) * 64)

    if C not in _NC_CACHE:
        _NC_CACHE[C] = build_nc(C)
    nc = _NC_CACHE[C]

    xfT_bf = np.ascontiguousarray(xf.T).astype(BF16_NP)   # [H, T]
    wg = np.asarray(wg, np.float32)
    wu = np.asarray(wu, np.float32)
    wd = np.asarray(wd, np.float32)
    swg = np.asarray(swg, np.float32)
    swu = np.asarray(swu, np.float32)
    swd = np.asarray(swd, np.float32)

    in_maps = []
    idx_r = []
    w_r = []
    for r in range(N_CORES):
        lo, hi = starts[r], starts[r + 1]
        idx = t_sorted[lo:hi]
        idx_r.append(idx)
        w_r.append(w_sorted[lo:hi])
        xg = np.zeros((H, C), dtype=BF16_NP)
        xg[:, :len(idx)] = xfT_bf[:, idx]
        g, q = divmod(r, 2)
        in_maps.append({
            "xr": _pack_x(xg),
            "xs": _pack_x(xfT_bf[:, g * TS:(g + 1) * TS]),
            "wg": np.ascontiguousarray(wg[r]).astype(BF16_NP),
            "wu": np.ascontiguousarray(wu[r]).astype(BF16_NP),
            "wd": np.ascontiguousarray(wd[r]).astype(BF16_NP),
            "sg": _pack_up_w(swg[:, q * I_R:(q + 1) * I_R]),
            "su": _pack_up_w(swu[:, q * I_R:(q + 1) * I_R]),
            "sd": np.ascontiguousarray(
                swd[q * I_R:(q + 1) * I_R, :]).astype(BF16_NP),
        })

    res = run_bass_kernel_spmd(nc, in_maps, core_ids=list(range(N_CORES)))
    LAST_RESULT = res

    # ---- host combine: shared partial sums + weighted routed scatter ----
    yT = np.zeros((H, T), np.float32)
    for g in range(4):
        np.add(res.results[2 * g]["ys"], res.results[2 * g + 1]["ys"],
               out=yT[:, g * TS:(g + 1) * TS])
    for r in range(N_CORES):
        n = len(idx_r[r])
        if n:
            yT[:, idx_r[r]] += (res.results[r]["yr"][:, :n]
                                * w_r[r][None, :].astype(np.float32))
    return np.ascontiguousarray(yT.T).reshape(B, S, H).astype(np.float32)


# revision 8
# speedup vs baseline: 1.3770x; 1.0115x over previous
"""Sparse expert-parallel MoE kernel for Trainium2 (8 NeuronCores).

Strategy (hardcoded for the nn_MoE problem: H=1024, E=8, top-k=2, I=1408,
shared-I=2816, T=2*2048=4096 tokens, f32 inputs):

- The gate (softmax top-2) is tiny (0.03% of FLOPs) and is evaluated on the
  host in float64; routing decisions match the f32 reference (min rank-2/3
  score gap for this problem's data is ~4e-5, far above f32 noise).
- Routed experts are EXPERT-PARALLEL with true top-2 sparsity: core r owns
  expert r and computes it only over the tokens routed to it (host-side
  gather -> padded capacity C, a multiple of 64).  This is ~4x fewer FLOPs
  than dense all-expert compute.
- The shared expert is sharded 4x2: cores are split into 4 token-groups of
  2; within a group each core owns a 1408-wide half of the 2816 shared
  intermediate dim (11 full 128-tiles -> no partial-tile waste).  Partials
  are summed on the host.
- Combine: host scatter-adds  w_e(t) * expert_e(x_t)  (f32) plus the shared
  partial sums.  No on-device collectives.
- All matmuls run in bf16 with f32 PSUM accumulation (host pre-casts).
- DMA layouts are chosen so every transfer has >=2KB contiguous bytes per
  partition (the DMA pays ~110ns per line descriptor): x inputs are packed
  [128, KC, ntok] and fully SBUF-resident; shared up-weights are packed
  [128, IT, KC*128] and DMA'd per intermediate tile in consumption order
  (first matmul group only waits for ~0.5 MB).
- The shared phase runs FIRST; the routed phase starts with its small
  64-token remainder chunk (so the inefficient small-line store overlaps
  compute) and ends on a full 512 chunk (short drain tail).

Layouts put features on the partition axis and tokens on the free axis:
    up:   hg[i, t] = sum_h wg[h, i] * xT[h, t]   (lhsT=wg nat., rhs=xT)
    down: eo[h, t] = sum_i wd[i, h] * act[i, t]  (lhsT=wd nat., rhs=act)
"""

import os
import sys

for _p in ("/opt/trn_rl_repo", "/root/.axon_site/_ro/trn_rl_repo"):
    if os.path.isdir(_p) and _p not in sys.path:
        sys.path.insert(0, _p)

import numpy as np

import concourse.bass as bass
import concourse.mybir as mybir
import concourse.tile as tile
from concourse import bacc
from concourse.bass_utils import run_bass_kernel_spmd

F32 = mybir.dt.float32
BF16 = mybir.dt.bfloat16
BF16_NP = mybir.dt.np(mybir.dt.bfloat16)
AX = mybir.AxisListType
ALU = mybir.AluOpType
ACTF = mybir.ActivationFunctionType

H = 1024           # hidden
E = 8              # experts = cores
TOP_K = 2
I_R = 1408         # routed intermediate = shared intermediate half (2816/2)
TS = 1024          # shared-expert tokens per core (4096 / 4 groups)
N_CORES = 8
KC = H // 128      # 8 contraction chunks over hidden
IT_R = I_R // 128  # 11 intermediate tiles (routed and shared-half alike)
TC = 512           # token tile (PSUM bank = 512 f32)

LAST_RESULT = None  # BassKernelResults of the most recent run (for profiling)


def _chunks_of(n, small_first=False):
    out = [TC] * (n // TC)
    if n % TC:
        if small_first:
            out.insert(0, n % TC)
        else:
            out.append(n % TC)
    return out


def build_nc(C, trace_sim=False, silu_via_sigmoid=False):
    """Build the SPMD Bass program (identical on all 8 cores).

    C: routed-token capacity per core (multiple of 64).
    silu_via_sigmoid: CoreSim has no Silu LUT; emulate as x*sigmoid(x).
    """
    nc = bacc.Bacc("TRN2", target_bir_lowering=False, debug=False,
                   num_devices=N_CORES)

    # x inputs packed [128, KC, ntok]: per-k DMA has ntok*2B contiguous/line
    xr = nc.dram_tensor("xr", [128, KC, C], BF16, kind="ExternalInput")
    xs = nc.dram_tensor("xs", [128, KC, TS], BF16, kind="ExternalInput")
    wg = nc.dram_tensor("wg", [H, I_R], BF16, kind="ExternalInput")
    wu = nc.dram_tensor("wu", [H, I_R], BF16, kind="ExternalInput")
    wd = nc.dram_tensor("wd", [I_R, H], BF16, kind="ExternalInput")
    # shared up-weights packed [128, IT, KC*128]: per-it DMA, 2KB lines
    sg = nc.dram_tensor("sg", [128, IT_R, KC * 128], BF16,
                        kind="ExternalInput")
    su = nc.dram_tensor("su", [128, IT_R, KC * 128], BF16,
                        kind="ExternalInput")
    sd = nc.dram_tensor("sd", [I_R, H], BF16, kind="ExternalInput")
    yr = nc.dram_tensor("yr", [H, C], F32, kind="ExternalOutput")
    ys = nc.dram_tensor("ys", [H, TS], F32, kind="ExternalOutput")

    with tile.TileContext(nc, trace_sim=trace_sim) as tc:
        with (
            tc.tile_pool(name="const", bufs=1) as cpool,
            tc.tile_pool(name="act", bufs=2) as actpool,
            tc.tile_pool(name="tmp", bufs=3) as tpool,
            tc.tile_pool(name="eo", bufs=3) as eopool,
            tc.tile_pool(name="ps_up", bufs=4, space="PSUM") as ps_up,
            tc.tile_pool(name="ps_o", bufs=2, space="PSUM") as ps_o,
        ):
            # ---- inputs in consumption order; the first psum group needs
            # sg/su tile 0 + xs, so those go first, each on its own DMA
            # queue (round-robin), before the bulk weight stream ----
            sg_ts, su_ts = [], []
            for it in range(IT_R):
                sgt = cpool.tile([128, KC, 128], BF16, tag=f"sg{it}")
                sut = cpool.tile([128, KC, 128], BF16, tag=f"su{it}")
                sg_ts.append(sgt)
                su_ts.append(sut)
            for it in range(2):
                nc.sync.dma_start(sg_ts[it][:, :, :], sg[:, it, :])
                nc.sync.dma_start(su_ts[it][:, :, :], su[:, it, :])
            xs_t = cpool.tile([128, KC, TS], BF16, tag="xs")
            for k in range(KC):
                nc.sync.dma_start(xs_t[:, k, :], xs[:, k, :])
            for it in range(2, IT_R):
                nc.sync.dma_start(sg_ts[it][:, :, :], sg[:, it, :])
                nc.sync.dma_start(su_ts[it][:, :, :], su[:, it, :])
            sd_ts = []
            for it in range(IT_R):
                sdt = cpool.tile([128, H], BF16, tag=f"sd{it}")
                nc.sync.dma_start(sdt[:, :], sd[it * 128:(it + 1) * 128, :])
                sd_ts.append(sdt)
            # routed inputs (needed ~120us in; stream behind shared ones)
            xr_t = cpool.tile([128, KC, C], BF16, tag="xr")
            for k in range(KC):
                nc.sync.dma_start(xr_t[:, k, :], xr[:, k, :])
            wg_ks, wu_ks = [], []
            for k in range(KC):
                wgk = cpool.tile([128, I_R], BF16, tag=f"wg{k}")
                nc.sync.dma_start(wgk[:, :], wg[k * 128:(k + 1) * 128, :])
                wuk = cpool.tile([128, I_R], BF16, tag=f"wu{k}")
                nc.sync.dma_start(wuk[:, :], wu[k * 128:(k + 1) * 128, :])
                wg_ks.append(wgk)
                wu_ks.append(wuk)
            wd_ts = []
            for it in range(IT_R):
                wdt = cpool.tile([128, H], BF16, tag=f"wd{it}")
                nc.sync.dma_start(wdt[:, :], wd[it * 128:(it + 1) * 128, :])
                wd_ts.append(wdt)

            def swiglu_chunk(x_t, t0, n, gate_f, up_f, act_t):
                """act[i, :n] = silu(gate) * up over tokens [t0, t0+n).

                gate_f/up_f: (it, k) -> lhsT [128, 128] weight block.
                """
                for it in range(IT_R):
                    pg = ps_up.tile([128, TC], F32, tag="up")
                    for k in range(KC):
                        nc.tensor.matmul(pg[:, :n], gate_f(it, k),
                                         x_t[:, k, t0:t0 + n],
                                         start=(k == 0), stop=(k == KC - 1))
                    pu = ps_up.tile([128, TC], F32, tag="up")
                    for k in range(KC):
                        nc.tensor.matmul(pu[:, :n], up_f(it, k),
                                         x_t[:, k, t0:t0 + n],
                                         start=(k == 0), stop=(k == KC - 1))
                    sa = tpool.tile([128, TC], F32, tag="sa")
                    if silu_via_sigmoid:
                        nc.scalar.activation(sa[:, :n], pg[:, :n],
                                             ACTF.Sigmoid)
                        nc.vector.tensor_mul(sa[:, :n], sa[:, :n], pg[:, :n])
                    else:
                        nc.scalar.activation(sa[:, :n], pg[:, :n], ACTF.Silu)
                    nc.vector.tensor_mul(act_t[:, it, :n], sa[:, :n],
                                         pu[:, :n])

            def down_chunk(act_t, n, down_ts, out_d, t0):
                for hc in range(KC):
                    h0 = hc * 128
                    po = ps_o.tile([128, TC], F32, tag="o")
                    for it in range(IT_R):
                        nc.tensor.matmul(
                            po[:, :n], down_ts[it][:, h0:h0 + 128],
                            act_t[:, it, :n], start=(it == 0),
                            stop=(it == IT_R - 1))
                    eo = eopool.tile([128, TC], F32)
                    nc.vector.tensor_copy(eo[:, :n], po[:, :n])
                    nc.sync.dma_start(out_d[h0:h0 + 128, t0:t0 + n], eo[:, :n])

            # ---- shared expert half over this core's token group ----
            t0 = 0
            for n in _chunks_of(TS):
                act_t = actpool.tile([128, IT_R, TC], BF16, tag="act")
                swiglu_chunk(xs_t, t0, n, lambda it, k: sg_ts[it][:, k, :],
                             lambda it, k: su_ts[it][:, k, :], act_t)
                down_chunk(act_t, n, sd_ts, ys, t0)
                t0 += n

            # ---- routed expert over gathered tokens (small chunk first:
            # its small-line store DMA overlaps later compute) ----
            t0 = 0
            for n in _chunks_of(C, small_first=True):
                act_t = actpool.tile([128, IT_R, TC], BF16, tag="act")
                swiglu_chunk(
                    xr_t, t0, n,
                    lambda it, k: wg_ks[k][:, it * 128:(it + 1) * 128],
                    lambda it, k: wu_ks[k][:, it * 128:(it + 1) * 128], act_t)
                down_chunk(act_t, n, wd_ts, yr, t0)
                t0 += n

    nc.compile()
    return nc


def _route_host(xf, gate_w):
    """Replicate the reference MoEGate exactly (float64 for determinism)."""
    logits = xf.astype(np.float64) @ gate_w.astype(np.float64).T
    m = logits.max(axis=-1, keepdims=True)
    ex = np.exp(logits - m)
    sc = ex / ex.sum(axis=-1, keepdims=True)
    topi = np.argsort(-sc, axis=-1, kind="stable")[:, :TOP_K]   # ties: low idx
    topw = np.take_along_axis(sc, topi, axis=-1)
    topw = topw / (topw.sum(axis=-1, keepdims=True) + 1e-20)    # SCALE = 1.0
    return topi, topw


def _pack_x(xT_bf):
    """[H, ntok] -> [128, KC, ntok] partition-major pack."""
    n = xT_bf.shape[1]
    return np.ascontiguousarray(
        xT_bf.reshape(KC, 128, n).transpose(1, 0, 2))


def _pack_up_w(w):
    """[H, I_R] -> [128, IT, KC*128] pack (2KB lines per it-tile DMA)."""
    a = np.ascontiguousarray(w).astype(BF16_NP)
    a = a.reshape(KC, 128, IT_R, 128).transpose(1, 2, 0, 3)
    return np.ascontiguousarray(a.reshape(128, IT_R, KC * 128))


_NC_CACHE = {}


def kernel(x, gate_w, wg, wu, wd, swg, swu, swd):
    global LAST_RESULT
    x = np.asarray(x, np.float32)
    B, S, _ = x.shape
    T = B * S
    xf = x.reshape(T, H)

    # ---- host gate + dispatch ----
    topi, topw = _route_host(xf, np.asarray(gate_w, np.float32))
    e_ids = topi.ravel()
    t_ids = np.repeat(np.arange(T), TOP_K)
    w_all = topw.ravel()
    order = np.argsort(e_ids, kind="stable")
    e_sorted = e_ids[order]
    t_sorted = t_ids[order]
    w_sorted = w_all[order]
    counts = np.bincount(e_sorted, minlength=E)
    starts = np.concatenate([[0], np.cumsum(counts)])
    C = max(64, int(-(-counts.max() // 64)) * 64)

    if C not in _NC_CACHE:
        _NC_CACHE[C] = build_nc(C)
    nc = _NC_CACHE[C]

    xfT_bf = np.ascontiguousarray(xf.T).astype(BF16_NP)   # [H, T]
    wg = np.asarray(wg, np.float32)
    wu = np.asarray(wu, np.float32)
    wd = np.asarray(wd, np.float32)
    swg = np.asarray(swg, np.float32)
    swu = np.asarray(swu, np.float32)
    swd = np.asarray(swd, np.float32)

    in_maps = []
    idx_r = []
    w_r = []
    for r in range(N_CORES):
        lo, hi = starts[r], starts[r + 1]
        idx = t_sorted[lo:hi]
        idx_r.append(idx)
        w_r.append(w_sorted[lo:hi])
        xg = np.zeros((H, C), dtype=BF16_NP)
        xg[:, :len(idx)] = xfT_bf[:, idx]
        g, q = divmod(r, 2)
        in_maps.append({
            "xr": _pack_x(xg),
            "xs": _pack_x(xfT_bf[:, g * TS:(g + 1) * TS]),
            "wg": np.ascontiguousarray(wg[r]).astype(BF16_NP),
            "wu": np.ascontiguousarray(wu[r]).astype(BF16_NP),
            "wd": np.ascontiguousarray(wd[r]).astype(BF16_NP),
            "sg": _pack_up_w(swg[:, q * I_R:(q + 1) * I_R]),
            "su": _pack_up_w(swu[:, q * I_R:(q + 1) * I_R]),
            "sd": np.ascontiguousarray(
                swd[q * I_R:(q + 1) * I_R, :]).astype(BF16_NP),
        })

    res = run_bass_kernel_spmd(nc, in_maps, core_ids=list(range(N_CORES)))
    LAST_RESULT = res

    # ---- host combine: shared partial sums + weighted routed scatter ----
    yT = np.zeros((H, T), np.float32)
    for g in range(4):
        np.add(res.results[2 * g]["ys"], res.results[2 * g + 1]["ys"],
               out=yT[:, g * TS:(g + 1) * TS])
    for r in range(N_CORES):
        n = len(idx_r[r])
        if n:
            yT[:, idx_r[r]] += (res.results[r]["yr"][:, :n]
                                * w_r[r][None, :].astype(np.float32))
    return np.ascontiguousarray(yT.T).reshape(B, S, H).astype(np.float32)
